# revision 1
# baseline (speedup 1.0000x reference)
"""Causal multi-head attention (16 heads, hd=64) on 8 trn2 NeuronCores.

Sharding: core c -> batch b = c // 4, head-group g = c % 4 (4 heads = 256
columns of Wq/Wk/Wv).  Each core computes its [S, 256] slice of the three
outputs (attn out, K_cache, V_cache); the host gathers slices.

Per-core pipeline (Tile framework), all matmuls in fp32r (full PE rate at
N>=256, ~1e-4 operand rounding):
  - xT [1024, S] is host-transposed x[b]; weights/biases host-sliced.
  - KT/QT [c, q] computed directly (lhsT = W chunk), per-partition bias
    added during the DVE eviction; K_cache leaves the chip in kt's [c, s]
    layout (contiguous DMA) and the host transposes it in the gather.
  - Vf natural [s, c] (rank-1 bias matmul) -> V_cache + V_aug tiles
    [k, 65] per head (ones column -> softmax denominator; ones written
    by DVE -- a strided sub-word DMA would RMW-race adjacent columns).
  - scores ST[k, q]: the two heads of a pair run as concurrent
    row-tiled matmuls (K=64, partition offsets 0/64) into one
    [128, 1024] psum tile; diagonal blocks narrowed to the valid q
    range; one exp per k-tile over both heads (ACT, scale=1/8, per-k
    pad bias), fill-0 affine_select on the 128-wide partial triangle.
  - AV: out_unnorm[65, q] += V_aug.T @ PT over k-tiles; PE transpose
    back to [q, 65]; divide by the ones-row sum (reciprocal +
    tensor_scalar_mul); assemble [128, 256] rows, DMA out.
  - Emission order interleaves projections with attention per q-slice so
    attention starts as soon as its k-range is projected.
"""

import numpy as np

P = 128
S = 2048
HIN = 1024
C = 256  # columns per core = 4 heads * 64
HD = 64
NCORES = 8
HC = HIN // P  # 8 contraction chunks
NKT = S // P  # 16 k-tiles
QW = 512  # q-slice width
NQ = S // QW  # 4 q-slices
NPAIR = C // P  # 2 head-pairs per core

_nc_cache = None


def build_nc():
    import concourse.bacc as bacc
    import concourse.mybir as mybir
    from concourse.tile import TileContext
    from concourse.masks import make_identity
    from contextlib import ExitStack

    f32 = mybir.dt.float32
    f32r = mybir.dt.float32r
    bf16 = mybir.dt.bfloat16
    Exp = mybir.ActivationFunctionType.Exp
    Identity = mybir.ActivationFunctionType.Identity
    is_ge = mybir.AluOpType.is_ge

    nc = bacc.Bacc(None, target_bir_lowering=False)

    xt = nc.declare_dram_parameter("xt", [HIN, S], f32r, isOutput=False)
    wq = nc.declare_dram_parameter("wq", [HIN, C], f32r, isOutput=False)
    wk = nc.declare_dram_parameter("wk", [HIN, C], f32r, isOutput=False)
    wv = nc.declare_dram_parameter("wv", [HIN, C], f32r, isOutput=False)
    bqc = nc.declare_dram_parameter("bqc", [P, NPAIR], f32, isOutput=False)
    bkc = nc.declare_dram_parameter("bkc", [P, NPAIR], f32, isOutput=False)
    bv = nc.declare_dram_parameter("bv", [1, C], f32r, isOutput=False)
    padneg = nc.declare_dram_parameter("padneg", [P, NKT], f32, isOutput=False)
    ones = nc.declare_dram_parameter("ones", [P, C], f32r, isOutput=False)
    out = nc.declare_dram_parameter("out", [S, C], f32, isOutput=True)
    kct = nc.declare_dram_parameter("kct", [C, S], f32, isOutput=True)
    vc = nc.declare_dram_parameter("vc", [S, C], f32, isOutput=True)

    with TileContext(nc) as tc, ExitStack() as ctx:
        persist = ctx.enter_context(tc.tile_pool(name="persist", bufs=1))
        xt_sb = persist.tile([P, HC, S], f32r)
        wq_sb = persist.tile([P, HC, C], f32r)
        wk_sb = persist.tile([P, HC, C], f32r)
        wv_sb = persist.tile([P, HC, C], f32r)
        bqc_sb = persist.tile([P, NPAIR], f32)
        bkc_sb = persist.tile([P, NPAIR], f32)
        bv_sb = persist.tile([1, C], f32r)
        pn_sb = persist.tile([P, NKT], f32)
        ones_sb = persist.tile([P, C], f32r)
        ident = persist.tile([P, P], f32)
        qt_bf = persist.tile([P, NPAIR, S], f32r)
        kt_sb = persist.tile([P, NPAIR, S], f32r)
        va_bf = persist.tile([P, NKT, NPAIR, 2 * (HD + 1)], f32r)
        ofin = persist.tile([P, NKT, C], f32)

        # small constants first, then interleave weights (SWDGE/gpsimd
        # queue) with x chunks (HWDGE/sync) so the chunk-j projection
        # matmuls can start as soon as chunk j has arrived
        nc.sync.dma_start(bqc_sb[:], bqc[:])
        nc.sync.dma_start(bkc_sb[:], bkc[:])
        nc.sync.dma_start(bv_sb[:], bv[:])
        nc.sync.dma_start(pn_sb[:], padneg[:])
        nc.sync.dma_start(ones_sb[:], ones[:])
        # weights per chunk on the gpsimd queue, x in quarter-pieces on
        # sync: fine-grained deps let chunk-j matmuls start at arrival
        quarter = S // 4
        for j in range(HC):
            nc.gpsimd.dma_start(wq_sb[:, j, :], wq[j * P : (j + 1) * P, :])
            nc.gpsimd.dma_start(wk_sb[:, j, :], wk[j * P : (j + 1) * P, :])
            nc.gpsimd.dma_start(wv_sb[:, j, :], wv[j * P : (j + 1) * P, :])
            for h in range(4):
                nc.sync.dma_start(
                    xt_sb[:, j, h * quarter : (h + 1) * quarter],
                    xt[j * P : (j + 1) * P, h * quarter : (h + 1) * quarter],
                )
        # ones columns of V_aug (positions 64 and 129).  Written by DVE, not
        # DMA: a strided sub-word DMA write would RMW-race the adjacent
        # DVE-written V columns.
        ones3 = ones_sb[:, : NKT * NPAIR].rearrange("p (a b) -> p a b", a=NKT)
        nc.vector.tensor_copy(
            out=va_bf[:, :, :, HD : HD + 1], in_=ones3[:, :, :, None]
        )
        nc.vector.tensor_copy(
            out=va_bf[:, :, :, 2 * HD + 1 : 2 * HD + 2], in_=ones3[:, :, :, None]
        )
        make_identity(nc, ident[:])

        psum = ctx.enter_context(tc.tile_pool(name="psum", bufs=2, space="PSUM"))
        work = ctx.enter_context(tc.tile_pool(name="work", bufs=3))

        def kt_qt_slice(qi):
            qsl = slice(qi * QW, (qi + 1) * QW)
            for p in range(NPAIR):
                csl = slice(p * P, (p + 1) * P)
                for w_sb, b_sb, dst, dt_out in (
                    (wk_sb, bkc_sb, None, None),
                    (wq_sb, bqc_sb, qt_bf, bf16),
                ):
                    ps = psum.tile([P, QW], f32, tag="proj", bufs=2, name="p_ps")
                    for j in range(HC):
                        nc.tensor.matmul(
                            ps, w_sb[:, j, csl], xt_sb[:, j, qsl],
                            start=(j == 0), stop=(j == HC - 1),
                        )
                    if dst is None:
                        nc.vector.tensor_scalar_add(
                            kt_sb[:, p, qsl], ps, b_sb[:, p : p + 1]
                        )
                    else:
                        nc.vector.tensor_scalar_add(
                            dst[:, p, qsl], ps, b_sb[:, p : p + 1]
                        )

        def v_wave(qi):
            for i in range(4 * qi, 4 * qi + 4):
                ksl = slice(i * P, (i + 1) * P)
                ps = psum.tile([P, QW], f32, tag="proj", bufs=2, name="v_ps")[:, :C]
                for j in range(HC):
                    nc.tensor.matmul(
                        ps, xt_sb[:, j, ksl], wv_sb[:, j, :],
                        start=(j == 0), stop=False,
                    )
                nc.tensor.matmul(
                    ps, ones_sb[:1, :P], bv_sb[:1, :], start=False, stop=True
                )
                sb = work.tile([P, C], f32, tag="projsb", bufs=4, name="v_sb")
                nc.any.tensor_copy(out=sb[:], in_=ps)
                nc.sync.dma_start(vc[ksl, :], sb[:])
                for p in range(NPAIR):
                    nc.vector.tensor_copy(
                        out=va_bf[:, i, p, 0:HD], in_=sb[:, p * P : p * P + HD]
                    )
                    nc.vector.tensor_copy(
                        out=va_bf[:, i, p, HD + 1 : 2 * HD + 1],
                        in_=sb[:, p * P + HD : (p + 1) * P],
                    )

        def attention(qi):
            for p in range(NPAIR):
                av_a = psum.tile([HD + 1, QW], f32, tag="av", bufs=2, name="av_a")
                av_b = psum.tile([HD + 1, QW], f32, tag="av", bufs=2, name="av_b")
                tmax = 4 * qi + 4
                for t in range(tmax):
                    ksl = slice(t * P, (t + 1) * P)
                    d = t - 4 * qi
                    W = QW if d < 0 else QW - d * P
                    q0 = qi * QW + (0 if d < 0 else d * P)
                    st = psum.tile([P, 2 * QW], f32, tag="st", bufs=2, name="st")
                    nc.tensor.matmul(
                        st[:, 0:W], kt_sb[0:HD, p, ksl],
                        qt_bf[0:HD, p, q0 : q0 + W], start=True, stop=True,
                    )
                    nc.tensor.matmul(
                        st[:, QW : QW + W], kt_sb[HD:P, p, ksl],
                        qt_bf[HD:P, p, q0 : q0 + W], start=True, stop=True,
                    )
                    pt = work.tile([P, 2, QW], f32r, tag="pt", bufs=4, name="pt")
                    st3 = st[:].rearrange("p (h w) -> p h w", h=2)[:, :, 0:W]
                    nc.scalar.activation(
                        pt[:, :, 0:W], st3, Exp, bias=pn_sb[:, t : t + 1],
                        scale=0.125,
                    )
                    if d >= 0:
                        nc.gpsimd.affine_select(
                            out=pt[:, :, 0:P], in_=pt[:, :, 0:P],
                            compare_op=is_ge, fill=0.0, base=0,
                            pattern=[[0, 2], [1, P]], channel_multiplier=-1,
                        )
                    nc.tensor.matmul(
                        av_a[:, QW - W :], va_bf[:, t, p, 0 : HD + 1],
                        pt[:, 0, 0:W], start=(t == 0), stop=(t == tmax - 1),
                    )
                    nc.tensor.matmul(
                        av_b[:, QW - W :], va_bf[:, t, p, HD + 1 : 2 * HD + 2],
                        pt[:, 1, 0:W], start=(t == 0), stop=(t == tmax - 1),
                    )
                for h, av in ((0, av_a), (1, av_b)):
                    osb = work.tile([HD + 1, QW], f32, tag="osb", bufs=3, name="osb")
                    nc.vector.tensor_copy(out=osb[:], in_=av)
                    for sub in range(4):
                        tr = psum.tile(
                            [P, P], f32, tag="av", bufs=2, name="otr"
                        )[:, : HD + 1]
                        nc.tensor.transpose(
                            tr, osb[:, sub * P : (sub + 1) * P],
                            ident[: HD + 1, : HD + 1],
                        )
                        rcp = work.tile([P, 1], f32, tag="rcp", bufs=2, name="rcp")
                        nc.vector.reciprocal(rcp[:], tr[:, HD : HD + 1])
                        i = 4 * qi + sub
                        col = p * P + h * HD
                        nc.vector.tensor_scalar_mul(
                            ofin[:, i, col : col + HD], tr[:, 0:HD], rcp[:]
                        )
            for sub in range(4):
                i = 4 * qi + sub
                nc.sync.dma_start(out[i * P : (i + 1) * P, :], ofin[:, i, :])

        # interleaved emission: project a q/k-slice, then run the attention
        # that only needs what's already projected
        for qi in range(NQ):
            kt_qt_slice(qi)
            v_wave(qi)
            attention(qi)
            # K_cache leaves the chip in kt's [c, s] layout (contiguous
            # DMA); the host transposes it during the gather
            for p in range(NPAIR):
                nc.sync.dma_start(
                    kct[p * P : (p + 1) * P, qi * QW : (qi + 1) * QW],
                    kt_sb[:, p, qi * QW : (qi + 1) * QW].bitcast(f32),
                )

    nc.finalize()
    return nc


def get_nc():
    global _nc_cache
    if _nc_cache is None:
        _nc_cache = build_nc()
    return _nc_cache


def make_in_maps(x, pad_mask, Wq, bq, Wk, bk, Wv, bv):
    x = np.asarray(x, np.float32)
    pad_mask = np.asarray(pad_mask, np.float32)
    Wq = np.asarray(Wq, np.float32)
    bq = np.asarray(bq, np.float32)
    Wk = np.asarray(Wk, np.float32)
    bk = np.asarray(bk, np.float32)
    Wv = np.asarray(Wv, np.float32)
    bv = np.asarray(bv, np.float32)
    in_maps = []
    for c in range(NCORES):
        b, g = divmod(c, 4)
        cols = slice(g * C, (g + 1) * C)
        xt = np.ascontiguousarray(x[b].T)  # [HIN, S]
        pn = ((pad_mask[b] - 1.0) * 1e6).reshape(NKT, P).T.copy()  # [P, NKT]
        in_maps.append(
            dict(
                xt=xt,
                ones=np.ones((P, C), np.float32),
                wq=np.ascontiguousarray(Wq[:, cols]),
                wk=np.ascontiguousarray(Wk[:, cols]),
                wv=np.ascontiguousarray(Wv[:, cols]),
                bqc=np.ascontiguousarray(bq[cols].reshape(NPAIR, P).T),
                bkc=np.ascontiguousarray(bk[cols].reshape(NPAIR, P).T),
                bv=np.ascontiguousarray(bv[cols].reshape(1, C)),
                padneg=pn,
            )
        )
    return in_maps


def gather(results):
    B = 2
    out = np.empty((B, S, HIN), np.float32)
    kcache = np.empty((B, S, HIN), np.float32)
    vcache = np.empty((B, S, HIN), np.float32)
    for c in range(NCORES):
        b, g = divmod(c, 4)
        cols = slice(g * C, (g + 1) * C)
        out[b, :, cols] = results[c]["out"]
        kcache[b, :, cols] = results[c]["kct"].T
        vcache[b, :, cols] = results[c]["vc"]
    return out, kcache, vcache


def kernel(x, pad_mask, Wq, bq, Wk, bk, Wv, bv):
    from concourse.bass_utils import run_bass_kernel_spmd

    nc = get_nc()
    in_maps = make_in_maps(x, pad_mask, Wq, bq, Wk, bk, Wv, bv)
    res = run_bass_kernel_spmd(nc, in_maps, list(range(NCORES)))
    return gather(res.results)



# revision 5
# speedup vs baseline: 1.3205x; 1.3205x over previous
"""Causal multi-head attention (16 heads, hd=64) on 8 trn2 NeuronCores.

Sharding: core c -> batch b = c // 4, head-group g = c % 4 (4 heads = 256
columns of Wq/Wk/Wv).  Each core computes its [S, 256] slice of the three
outputs (attn out, K_cache, V_cache); the host gathers slices.

v2 vs baseline (205us):
  - bf16 end-to-end: host casts x/W to bf16 (halves input DMA + SBUF),
    K/Q/V/probs kept bf16 on-chip (enables PE fast-weight-load), outputs
    DMA'd bf16 and upcast on the host (halves output DMA).  PSUM math
    stays fp32.
  - x DMA emitted k-range-major (all 8 contraction chunks of the first
    512 keys first) so the first projection finishes ~8us in instead of
    ~45us.
  - software-pipelined attention: AV(t) is emitted one iteration late so
    scores(t+1) never sits behind it in the PE FIFO, and projection
    matmuls of the NEXT q-slice are interleaved as PE filler while the
    Scalar engine runs exp - keeps the PE HAM-warm (2.4 GHz).
  - V eviction writes the packed V_aug layout with one strided DVE copy;
    normalization does one reciprocal per 4 row-tiles + one broadcasted
    multiply.
"""

import numpy as np

P = 128
S = 2048
HIN = 1024
C = 256  # columns per core = 4 heads * 64
HD = 64
NCORES = 8
HC = HIN // P  # 8 contraction chunks
NKT = S // P  # 16 k-tiles
QW = 512  # q-slice width
NQ = S // QW  # 4 q-slices
NPAIR = C // P  # 2 head-pairs per core

_nc_cache = None


def build_nc():
    import concourse.bacc as bacc
    import concourse.mybir as mybir
    from concourse.tile import TileContext
    from concourse.masks import make_identity
    from contextlib import ExitStack
    from collections import deque

    f32 = mybir.dt.float32
    bf16 = mybir.dt.bfloat16
    Exp = mybir.ActivationFunctionType.Exp
    is_ge = mybir.AluOpType.is_ge
    bypass = mybir.AluOpType.bypass
    mult = mybir.AluOpType.mult

    nc = bacc.Bacc(None, target_bir_lowering=False)

    xt = nc.declare_dram_parameter("xt", [HIN, S], bf16, isOutput=False)
    wq = nc.declare_dram_parameter("wq", [HIN, C], bf16, isOutput=False)
    wk = nc.declare_dram_parameter("wk", [HIN, C], bf16, isOutput=False)
    wv = nc.declare_dram_parameter("wv", [HIN, C], bf16, isOutput=False)
    bqc = nc.declare_dram_parameter("bqc", [P, NPAIR], f32, isOutput=False)
    bkc = nc.declare_dram_parameter("bkc", [P, NPAIR], f32, isOutput=False)
    bv = nc.declare_dram_parameter("bv", [1, C], bf16, isOutput=False)
    padneg = nc.declare_dram_parameter("padneg", [P, NKT], f32, isOutput=False)
    out = nc.declare_dram_parameter("out", [S, C], bf16, isOutput=True)
    kct = nc.declare_dram_parameter("kct", [C, S], bf16, isOutput=True)
    vc = nc.declare_dram_parameter("vc", [S, C], bf16, isOutput=True)

    with TileContext(nc) as tc, ExitStack() as ctx:
        persist = ctx.enter_context(tc.tile_pool(name="persist", bufs=1))
        xt_sb = persist.tile([P, HC, S], bf16)
        wq_sb = persist.tile([P, HC, C], bf16)
        wk_sb = persist.tile([P, HC, C], bf16)
        wv_sb = persist.tile([P, HC, C], bf16)
        bqc_sb = persist.tile([P, NPAIR], f32)
        bkc_sb = persist.tile([P, NPAIR], f32)
        bv_sb = persist.tile([1, C], bf16)
        pn_sb = persist.tile([P, NKT], f32)
        one_sb = persist.tile([1, P], bf16)
        ident = persist.tile([P, P], f32)
        qt_bf = persist.tile([P, NPAIR, S], bf16)
        kt_sb = persist.tile([P, NPAIR, S], bf16)
        va_bf = persist.tile([P, NKT, NPAIR, 2 * (HD + 1)], bf16)
        ofin = persist.tile([P, NKT, C], bf16)

        nc.sync.dma_start(bqc_sb[:], bqc[:])
        nc.sync.dma_start(bkc_sb[:], bkc[:])
        nc.sync.dma_start(bv_sb[:], bv[:])
        nc.sync.dma_start(pn_sb[:], padneg[:])
        # ones + identity generated on-chip (no DMA)
        nc.vector.memset(one_sb[:], 1.0)
        nc.vector.memset(va_bf[:, :, :, HD : HD + 1], 1.0)
        nc.vector.memset(va_bf[:, :, :, 2 * HD + 1 : 2 * HD + 2], 1.0)
        make_identity(nc, ident[:])

        # weights on the gpsimd (SWDGE) queue; x on sync (HWDGE),
        # k-range-major so the first q/k-slice's full contraction is
        # resident early.
        quarter = S // 4
        for j in range(HC):
            nc.gpsimd.dma_start(wq_sb[:, j, :], wq[j * P : (j + 1) * P, :])
            nc.gpsimd.dma_start(wk_sb[:, j, :], wk[j * P : (j + 1) * P, :])
            nc.gpsimd.dma_start(wv_sb[:, j, :], wv[j * P : (j + 1) * P, :])
        for h in range(4):
            for j in range(HC):
                nc.sync.dma_start(
                    xt_sb[:, j, h * quarter : (h + 1) * quarter],
                    xt[j * P : (j + 1) * P, h * quarter : (h + 1) * quarter],
                )

        psum = ctx.enter_context(tc.tile_pool(name="psum", bufs=2, space="PSUM"))
        work = ctx.enter_context(tc.tile_pool(name="work", bufs=3))

        # ---- deferred-atom machinery: attention iterations pop proj/post
        # atoms as PE filler while the Scalar engine runs exp ----
        pending = deque()

        def fill(n):
            for _ in range(n):
                if not pending:
                    return
                pending.popleft()()

        def drain():
            while pending:
                pending.popleft()()

        def proj_atoms(qi):
            """kt/qt/v projections for q/k-slice qi, split into ~2-matmul
            atoms.  Returns a list of closures."""
            atoms = []
            qsl = slice(qi * QW, (qi + 1) * QW)
            for p in range(NPAIR):
                csl = slice(p * P, (p + 1) * P)
                for w_sb, b_sb, dst in (
                    (wk_sb, bkc_sb, kt_sb),
                    (wq_sb, bqc_sb, qt_bf),
                ):
                    cell = {}

                    def a_mm(j0, cell=cell, w_sb=w_sb, csl=csl, qsl=qsl):
                        if j0 == 0:
                            cell["ps"] = psum.tile(
                                [P, QW], f32, tag="proj", bufs=2, name="p_ps"
                            )
                        for j in (j0, j0 + 1):
                            nc.tensor.matmul(
                                cell["ps"], w_sb[:, j, csl], xt_sb[:, j, qsl],
                                start=(j == 0), stop=(j == HC - 1),
                            )

                    def a_ev(cell=cell, b_sb=b_sb, dst=dst, p=p, qsl=qsl, qi=qi):
                        nc.vector.tensor_scalar_add(
                            dst[:, p, qsl], cell["ps"], b_sb[:, p : p + 1]
                        )
                        if dst is kt_sb:
                            nc.sync.dma_start(
                                kct[p * P : (p + 1) * P, qsl], dst[:, p, qsl]
                            )

                    for j0 in range(0, HC, 2):
                        atoms.append(lambda j0=j0, f=a_mm: f(j0))
                    atoms.append(a_ev)
            for i in range(4 * qi, 4 * qi + 4):
                ksl = slice(i * P, (i + 1) * P)
                cell = {}

                def v_mm(j0, cell=cell, ksl=ksl):
                    if j0 == 0:
                        cell["ps"] = psum.tile(
                            [P, QW], f32, tag="proj", bufs=2, name="v_ps"
                        )[:, :C]
                    for j in (j0, j0 + 1):
                        nc.tensor.matmul(
                            cell["ps"], xt_sb[:, j, ksl], wv_sb[:, j, :],
                            start=(j == 0), stop=False,
                        )

                def v_ev(cell=cell, ksl=ksl, i=i):
                    nc.tensor.matmul(
                        cell["ps"], one_sb[:1, :P], bv_sb[:1, :],
                        start=False, stop=True,
                    )
                    # one strided DVE eviction into the packed V_aug
                    # layout [.., {V_h0, 1, V_h1, 1}]; vc leaves from the
                    # same tile via strided DMA read.
                    dst = va_bf[:, i, :, :].rearrange(
                        "p a (b c) -> p a b c", b=2, c=HD + 1
                    )[:, :, :, 0:HD]
                    src = cell["ps"].rearrange(
                        "p (a b c) -> p a b c", a=NPAIR, b=2
                    )
                    nc.vector.tensor_copy(out=dst, in_=src)
                    nc.sync.dma_start(vc[ksl, :], dst)

                for j0 in range(0, HC, 2):
                    atoms.append(lambda j0=j0, f=v_mm: f(j0))
                atoms.append(v_ev)
            return atoms

        def post_atoms(qi, p, av_a, av_b):
            """Normalize + transpose the finished AV psums of pair p."""
            atoms = []
            for h, av in ((0, av_a), (1, av_b)):
                cell = {}

                def a_osb(cell=cell, av=av):
                    cell["osb"] = work.tile(
                        [HD + 1, QW], f32, tag="osb", bufs=3, name="osb"
                    )
                    nc.vector.tensor_copy(out=cell["osb"][:], in_=av)

                def a_tr(cell=cell, qi=qi, p=p, h=h):
                    # stride 66 keeps each transpose output 8B-aligned
                    tr = psum.tile(
                        [P, 4 * (HD + 2)], f32, tag="av", bufs=2, name="tr"
                    )
                    tr3 = tr[:].rearrange("p (s c) -> p s c", s=4)
                    for s in range(4):
                        nc.tensor.transpose(
                            tr3[:, s, 0 : HD + 1],
                            cell["osb"][:, s * P : (s + 1) * P],
                            ident[: HD + 1, : HD + 1],
                        )
                    rcp4 = work.tile([P, 4], f32, tag="rcp", bufs=2, name="rcp")
                    nc.vector.reciprocal(rcp4[:], tr3[:, :, HD : HD + 1])
                    col = p * P + h * HD
                    nc.vector.scalar_tensor_tensor(
                        out=ofin[:, 4 * qi : 4 * qi + 4, col : col + HD],
                        in0=tr3[:, :, 0:HD],
                        scalar=1.0,
                        in1=rcp4[:][:, :, None].broadcast_to([P, 4, HD]),
                        op0=bypass,
                        op1=mult,
                    )

                atoms.append(a_osb)
                atoms.append(a_tr)
            return atoms

        def out_dma_atoms(qi):
            atoms = []
            for sub in range(4):
                i = 4 * qi + sub

                def a(i=i):
                    nc.sync.dma_start(out[i * P : (i + 1) * P, :], ofin[:, i, :])

                atoms.append(a)
            return atoms

        def attention(qi):
            tmax = 4 * qi + 4
            iters_left = [2 * tmax]
            for p in range(NPAIR):
                av_a = psum.tile([HD + 1, QW], f32, tag="av", bufs=2, name="av_a")
                av_b = psum.tile([HD + 1, QW], f32, tag="av", bufs=2, name="av_b")
                pend_av = None
                for t in range(tmax):
                    ksl = slice(t * P, (t + 1) * P)
                    d = t - 4 * qi
                    W = QW if d < 0 else QW - d * P
                    q0 = qi * QW + (0 if d < 0 else d * P)
                    st = psum.tile([P, 2 * QW], f32, tag="st", bufs=2, name="st")
                    nc.tensor.matmul(
                        st[:, 0:W], kt_sb[0:HD, p, ksl],
                        qt_bf[0:HD, p, q0 : q0 + W], start=True, stop=True,
                    )
                    nc.tensor.matmul(
                        st[:, QW : QW + W], kt_sb[HD:P, p, ksl],
                        qt_bf[HD:P, p, q0 : q0 + W], start=True, stop=True,
                    )
                    pt = work.tile([P, 2, QW], bf16, tag="pt", bufs=4, name="pt")
                    st3 = st[:].rearrange("p (h w) -> p h w", h=2)[:, :, 0:W]
                    nc.scalar.activation(
                        pt[:, :, 0:W], st3, Exp, bias=pn_sb[:, t : t + 1],
                        scale=0.125,
                    )
                    if d >= 0:
                        nc.gpsimd.affine_select(
                            out=pt[:, :, 0:P], in_=pt[:, :, 0:P],
                            compare_op=is_ge, fill=0.0, base=0,
                            pattern=[[0, 2], [1, P]], channel_multiplier=-1,
                        )
                    if pend_av is not None:
                        pend_av()

                    def mk_av(t=t, W=W, pt=pt, av_a=av_a, av_b=av_b):
                        nc.tensor.matmul(
                            av_a[:, QW - W :],
                            va_bf[:, t, p, 0 : HD + 1],
                            pt[:, 0, 0:W], start=(t == 0), stop=(t == tmax - 1),
                        )
                        nc.tensor.matmul(
                            av_b[:, QW - W :],
                            va_bf[:, t, p, HD + 1 : 2 * HD + 2],
                            pt[:, 1, 0:W], start=(t == 0), stop=(t == tmax - 1),
                        )

                    pend_av = mk_av
                    iters_left[0] -= 1
                    if pending:
                        fill(-(-len(pending) // max(iters_left[0], 1)))
                pend_av()
                pending.extend(post_atoms(qi, p, av_a, av_b))
            pending.extend(out_dma_atoms(qi))

        # ---- emission: proj(0) straight, then attention(qi) with
        # proj(qi+1) interleaved as filler ----
        for a in proj_atoms(0):
            a()
        for qi in range(NQ):
            if qi + 1 < NQ:
                pending.extend(proj_atoms(qi + 1))
            attention(qi)
            drain()

    nc.finalize()
    return nc


def get_nc():
    global _nc_cache
    if _nc_cache is None:
        _nc_cache = build_nc()
    return _nc_cache


def make_in_maps(x, pad_mask, Wq, bq, Wk, bk, Wv, bv):
    import ml_dtypes

    bf16 = ml_dtypes.bfloat16
    x = np.asarray(x, np.float32)
    pad_mask = np.asarray(pad_mask, np.float32)
    Wq = np.asarray(Wq, np.float32)
    bq = np.asarray(bq, np.float32)
    Wk = np.asarray(Wk, np.float32)
    bk = np.asarray(bk, np.float32)
    Wv = np.asarray(Wv, np.float32)
    bv = np.asarray(bv, np.float32)
    in_maps = []
    for c in range(NCORES):
        b, g = divmod(c, 4)
        cols = slice(g * C, (g + 1) * C)
        xt = np.ascontiguousarray(x[b].T).astype(bf16)  # [HIN, S]
        pn = ((pad_mask[b] - 1.0) * 1e6).reshape(NKT, P).T.copy()  # [P, NKT]
        in_maps.append(
            dict(
                xt=xt,
                wq=np.ascontiguousarray(Wq[:, cols]).astype(bf16),
                wk=np.ascontiguousarray(Wk[:, cols]).astype(bf16),
                wv=np.ascontiguousarray(Wv[:, cols]).astype(bf16),
                bqc=np.ascontiguousarray(bq[cols].reshape(NPAIR, P).T),
                bkc=np.ascontiguousarray(bk[cols].reshape(NPAIR, P).T),
                bv=np.ascontiguousarray(bv[cols].reshape(1, C)).astype(bf16),
                padneg=pn,
            )
        )
    return in_maps


def gather(results):
    B = 2
    out = np.empty((B, S, HIN), np.float32)
    kcache = np.empty((B, S, HIN), np.float32)
    vcache = np.empty((B, S, HIN), np.float32)
    for c in range(NCORES):
        b, g = divmod(c, 4)
        cols = slice(g * C, (g + 1) * C)
        out[b, :, cols] = results[c]["out"].astype(np.float32)
        kcache[b, :, cols] = results[c]["kct"].T.astype(np.float32)
        vcache[b, :, cols] = results[c]["vc"].astype(np.float32)
    return out, kcache, vcache


def kernel(x, pad_mask, Wq, bq, Wk, bk, Wv, bv):
    from concourse.bass_utils import run_bass_kernel_spmd

    nc = get_nc()
    in_maps = make_in_maps(x, pad_mask, Wq, bq, Wk, bk, Wv, bv)
    res = run_bass_kernel_spmd(nc, in_maps, list(range(NCORES)))
    return gather(res.results)


# revision 8
# speedup vs baseline: 1.3207x; 1.0001x over previous
"""Causal multi-head attention (16 heads, hd=64) on 8 trn2 NeuronCores.

Sharding: core c -> batch b = c // 4, head-group g = c % 4 (4 heads = 256
columns of Wq/Wk/Wv).  Each core computes its [S, 256] slice of the three
outputs (attn out, K_cache, V_cache); the host gathers slices.

v3 (from v2 @160us, baseline 205us):
  - bf16 end-to-end (host casts x/W; outputs bf16, host upcasts); PSUM
    math fp32.
  - x DMA k-range-major; weights on the Vector DMA queue (GpSimd's
    SWDGE descriptor issue was blocking affine_select in the ramp);
    identity/memsets emitted first.
  - V bias folded into the DVE eviction (partition-broadcast bv once)
    instead of 16 rank-1 PE matmuls.
  - software-pipelined attention (AV deferred one iter) with a
    carry/fill scheduler: next q-slice's projections fill the PE while
    the Scalar engine runs exp, and late V-tiles + normalization posts
    are carried into the NEXT attention's iters so the ACT-bound tail
    still has PE work.
"""

import numpy as np

P = 128
S = 2048
HIN = 1024
C = 256  # columns per core = 4 heads * 64
HD = 64
NCORES = 8
HC = HIN // P  # 8 contraction chunks
NKT = S // P  # 16 k-tiles
QW = 512  # q-slice width
NQ = S // QW  # 4 q-slices
NPAIR = C // P  # 2 head-pairs per core

_nc_cache = None


def build_nc():
    import concourse.bacc as bacc
    import concourse.mybir as mybir
    from concourse.tile import TileContext
    from concourse.masks import make_identity
    from contextlib import ExitStack
    from collections import deque

    f32 = mybir.dt.float32
    bf16 = mybir.dt.bfloat16
    Exp = mybir.ActivationFunctionType.Exp
    is_ge = mybir.AluOpType.is_ge
    bypass = mybir.AluOpType.bypass
    mult = mybir.AluOpType.mult
    add = mybir.AluOpType.add

    nc = bacc.Bacc(None, target_bir_lowering=False)

    xt = nc.declare_dram_parameter("xt", [HIN, S], bf16, isOutput=False)
    wq = nc.declare_dram_parameter("wq", [HIN, C], bf16, isOutput=False)
    wk = nc.declare_dram_parameter("wk", [HIN, C], bf16, isOutput=False)
    wv = nc.declare_dram_parameter("wv", [HIN, C], bf16, isOutput=False)
    bqc = nc.declare_dram_parameter("bqc", [P, NPAIR], f32, isOutput=False)
    bkc = nc.declare_dram_parameter("bkc", [P, NPAIR], f32, isOutput=False)
    bv = nc.declare_dram_parameter("bv", [1, C], bf16, isOutput=False)
    padneg = nc.declare_dram_parameter("padneg", [P, NKT], f32, isOutput=False)
    out = nc.declare_dram_parameter("out", [S, C], bf16, isOutput=True)
    kct = nc.declare_dram_parameter("kct", [C, S], bf16, isOutput=True)
    vc = nc.declare_dram_parameter("vc", [S, C], bf16, isOutput=True)

    with TileContext(nc) as tc, ExitStack() as ctx:
        persist = ctx.enter_context(tc.tile_pool(name="persist", bufs=1))
        xt_sb = persist.tile([P, HC, S], bf16)
        wq_sb = persist.tile([P, HC, C], bf16)
        wk_sb = persist.tile([P, HC, C], bf16)
        wv_sb = persist.tile([P, HC, C], bf16)
        bqc_sb = persist.tile([P, NPAIR], f32)
        bkc_sb = persist.tile([P, NPAIR], f32)
        bv_sb = persist.tile([1, C], bf16)
        bvb_sb = persist.tile([P, C], bf16)
        pn_sb = persist.tile([P, NKT], f32)
        ident = persist.tile([P, P], f32)
        qt_bf = persist.tile([P, NPAIR, S], bf16)
        kt_sb = persist.tile([P, NPAIR, S], bf16)
        va_bf = persist.tile([P, NKT, NPAIR, 2 * (HD + 1)], bf16)
        ofin = persist.tile([P, NKT, C], bf16)

        # on-chip constants first so gpsimd/vector are free later
        nc.vector.memset(va_bf[:, :, :, HD : HD + 1], 1.0)
        nc.vector.memset(va_bf[:, :, :, 2 * HD + 1 : 2 * HD + 2], 1.0)
        make_identity(nc, ident[:])
        nc.sync.dma_start(bqc_sb[:], bqc[:])
        nc.sync.dma_start(bkc_sb[:], bkc[:])
        nc.sync.dma_start(bv_sb[:], bv[:])
        nc.sync.dma_start(pn_sb[:], padneg[:])
        nc.gpsimd.partition_broadcast(bvb_sb[:], bv_sb[:1, :])

        # weights on the vector DMA queue (idle in the ramp), x on sync
        # (HWDGE), k-range-major so the first 512-key slice's full
        # contraction is resident early.
        quarter = S // 4
        for j in range(HC):
            nc.scalar.dma_start(wq_sb[:, j, :], wq[j * P : (j + 1) * P, :])
            nc.scalar.dma_start(wk_sb[:, j, :], wk[j * P : (j + 1) * P, :])
            nc.scalar.dma_start(wv_sb[:, j, :], wv[j * P : (j + 1) * P, :])
        for h in range(4):
            for j in range(HC):
                nc.sync.dma_start(
                    xt_sb[:, j, h * quarter : (h + 1) * quarter],
                    xt[j * P : (j + 1) * P, h * quarter : (h + 1) * quarter],
                )

        psum = ctx.enter_context(tc.tile_pool(name="psum", bufs=2, space="PSUM"))
        work = ctx.enter_context(tc.tile_pool(name="work", bufs=3))

        def kq_atoms(qi):
            """kt/qt projections for q-slice qi as ~2-matmul atoms."""
            atoms = []
            qsl = slice(qi * QW, (qi + 1) * QW)
            for p in range(NPAIR):
                csl = slice(p * P, (p + 1) * P)
                for w_sb, b_sb, dst in (
                    (wk_sb, bkc_sb, kt_sb),
                    (wq_sb, bqc_sb, qt_bf),
                ):
                    cell = {}

                    def a_mm(j0, cell=cell, w_sb=w_sb, csl=csl, qsl=qsl):
                        if j0 == 0:
                            cell["ps"] = psum.tile(
                                [P, QW], f32, tag="proj", bufs=2, name="p_ps"
                            )
                        for j in (j0, j0 + 1):
                            nc.tensor.matmul(
                                cell["ps"], w_sb[:, j, csl], xt_sb[:, j, qsl],
                                start=(j == 0), stop=(j == HC - 1),
                            )

                    def a_ev(cell=cell, b_sb=b_sb, dst=dst, p=p, qsl=qsl):
                        nc.vector.tensor_scalar_add(
                            dst[:, p, qsl], cell["ps"], b_sb[:, p : p + 1]
                        )
                        if dst is kt_sb:
                            nc.sync.dma_start(
                                kct[p * P : (p + 1) * P, qsl], dst[:, p, qsl]
                            )

                    for j0 in range(0, HC, 2):
                        atoms.append(lambda j0=j0, f=a_mm: f(j0))
                    atoms.append(a_ev)
            return atoms

        def v_atoms(qi):
            """V projections for k-tiles 4qi..4qi+3; 5 atoms per tile."""
            atoms = []
            for i in range(4 * qi, 4 * qi + 4):
                ksl = slice(i * P, (i + 1) * P)
                cell = {}

                def v_mm(j0, cell=cell, ksl=ksl):
                    if j0 == 0:
                        cell["ps"] = psum.tile(
                            [P, QW], f32, tag="proj", bufs=2, name="v_ps"
                        )[:, :C]
                    for j in (j0, j0 + 1):
                        nc.tensor.matmul(
                            cell["ps"], xt_sb[:, j, ksl], wv_sb[:, j, :],
                            start=(j == 0), stop=(j == HC - 1),
                        )

                def v_ev(cell=cell, ksl=ksl, i=i):
                    # bias-add + eviction into the packed V_aug layout
                    # [.., {V_h0, 1, V_h1, 1}] in one strided DVE op; vc
                    # leaves from the same tile via strided DMA read.
                    dst = va_bf[:, i, :, :].rearrange(
                        "p a (b c) -> p a b c", b=2, c=HD + 1
                    )[:, :, :, 0:HD]
                    src = cell["ps"].rearrange(
                        "p (a b c) -> p a b c", a=NPAIR, b=2
                    )
                    bsrc = bvb_sb[:].rearrange(
                        "p (a b c) -> p a b c", a=NPAIR, b=2
                    )
                    nc.vector.tensor_tensor(out=dst, in0=src, in1=bsrc, op=add)
                    nc.sync.dma_start(vc[ksl, :], dst)

                for j0 in range(0, HC, 2):
                    atoms.append(lambda j0=j0, f=v_mm: f(j0))
                atoms.append(v_ev)
            return atoms

        def post_atoms(qi, p, av_a, av_b):
            """Normalize + transpose the finished AV psums of pair p."""
            atoms = []
            for h, av in ((0, av_a), (1, av_b)):
                cell = {}

                def a_osb(cell=cell, av=av):
                    cell["osb"] = work.tile(
                        [HD + 1, QW], f32, tag="osb", bufs=3, name="osb"
                    )
                    nc.vector.tensor_copy(out=cell["osb"][:], in_=av)

                def a_tr(cell=cell, qi=qi, p=p, h=h):
                    # stride 66 keeps each transpose output 8B-aligned
                    tr = psum.tile(
                        [P, 4 * (HD + 2)], f32, tag="av", bufs=2, name="tr"
                    )
                    tr3 = tr[:].rearrange("p (s c) -> p s c", s=4)
                    for s in range(4):
                        nc.tensor.transpose(
                            tr3[:, s, 0 : HD + 1],
                            cell["osb"][:, s * P : (s + 1) * P],
                            ident[: HD + 1, : HD + 1],
                        )
                    rcp4 = work.tile([P, 4], f32, tag="rcp", bufs=2, name="rcp")
                    nc.vector.reciprocal(rcp4[:], tr3[:, :, HD : HD + 1])
                    col = p * P + h * HD
                    nc.vector.scalar_tensor_tensor(
                        out=ofin[:, 4 * qi : 4 * qi + 4, col : col + HD],
                        in0=tr3[:, :, 0:HD],
                        scalar=1.0,
                        in1=rcp4[:][:, :, None].broadcast_to([P, 4, HD]),
                        op0=bypass,
                        op1=mult,
                    )

                atoms.append(a_osb)
                atoms.append(a_tr)
            return atoms

        def out_dma_atoms(qi):
            atoms = []
            for sub in range(4):
                i = 4 * qi + sub

                def a(i=i):
                    nc.sync.dma_start(out[i * P : (i + 1) * P, :], ofin[:, i, :])

                atoms.append(a)
            return atoms

        def attention(qi, pend, carry_v_tiles):
            """pend: deque of filler atoms; its first carry_v_tiles*5
            atoms are the held-over V projections for k-tiles
            4qi+2/4qi+3 and must be emitted before AV reads them."""
            tmax = 4 * qi + 4
            iters_left = [2 * tmax]
            v_left = [carry_v_tiles * 5]

            def fill():
                k = -(-len(pend) // max(iters_left[0], 1))
                for _ in range(k):
                    if not pend:
                        return
                    if v_left[0] > 0:
                        v_left[0] -= 1
                    pend.popleft()()

            def force_v(t):
                # v-tile 4qi+2+n must be fully emitted before av(t) with
                # t == 4qi+2+n is emitted (av(t) emits at iter t+1)
                need = min(carry_v_tiles * 5, max(0, (t - (4 * qi + 2) + 1)) * 5)
                done = carry_v_tiles * 5 - v_left[0]
                while done < need:
                    pend.popleft()()
                    v_left[0] -= 1
                    done += 1

            for p in range(NPAIR):
                av_a = psum.tile([HD + 1, QW], f32, tag="av", bufs=2, name="av_a")
                av_b = psum.tile([HD + 1, QW], f32, tag="av", bufs=2, name="av_b")
                pend_av = None
                for t in range(tmax):
                    ksl = slice(t * P, (t + 1) * P)
                    d = t - 4 * qi
                    W = QW if d < 0 else QW - d * P
                    q0 = qi * QW + (0 if d < 0 else d * P)
                    st = psum.tile([P, 2 * QW], f32, tag="st", bufs=2, name="st")
                    nc.tensor.matmul(
                        st[:, 0:W], kt_sb[0:HD, p, ksl],
                        qt_bf[0:HD, p, q0 : q0 + W], start=True, stop=True,
                    )
                    nc.tensor.matmul(
                        st[:, QW : QW + W], kt_sb[HD:P, p, ksl],
                        qt_bf[HD:P, p, q0 : q0 + W], start=True, stop=True,
                    )
                    pt = work.tile([P, 2, QW], bf16, tag="pt", bufs=4, name="pt")
                    st3 = st[:].rearrange("p (h w) -> p h w", h=2)[:, :, 0:W]
                    nc.scalar.activation(
                        pt[:, :, 0:W], st3, Exp, bias=pn_sb[:, t : t + 1],
                        scale=0.125,
                    )
                    if d >= 0:
                        nc.gpsimd.affine_select(
                            out=pt[:, :, 0:P], in_=pt[:, :, 0:P],
                            compare_op=is_ge, fill=0.0, base=0,
                            pattern=[[0, 2], [1, P]], channel_multiplier=-1,
                        )
                    if pend_av is not None:
                        pend_av()

                    if p == 0:
                        force_v(t)

                    def mk_av(t=t, W=W, pt=pt, av_a=av_a, av_b=av_b, p=p):
                        nc.tensor.matmul(
                            av_a[:, QW - W :],
                            va_bf[:, t, p, 0 : HD + 1],
                            pt[:, 0, 0:W], start=(t == 0), stop=(t == tmax - 1),
                        )
                        nc.tensor.matmul(
                            av_b[:, QW - W :],
                            va_bf[:, t, p, HD + 1 : 2 * HD + 2],
                            pt[:, 1, 0:W], start=(t == 0), stop=(t == tmax - 1),
                        )

                    pend_av = mk_av
                    iters_left[0] -= 1
                    fill()
                pend_av()
                if p == 0:
                    pend.extend(post_atoms(qi, p, av_a, av_b))
                else:
                    return post_atoms(qi, p, av_a, av_b)

        # ---- emission schedule ----
        for a in kq_atoms(0) + v_atoms(0):
            a()
        carry = []  # [v-tile atoms (carry_v tiles), posts, dmas] for next attn
        carry_v = 0
        for qi in range(NQ):
            pend = deque(carry)
            cv = carry_v
            if qi + 1 < NQ:
                v_nxt = v_atoms(qi + 1)
                pend.extend(kq_atoms(qi + 1))
                pend.extend(v_nxt[:10])  # k-tiles 4(qi+1)+0,1
                carry = v_nxt[10:]  # k-tiles 4(qi+1)+2,3 fill next attn
                carry_v = 2
            else:
                carry = []
                carry_v = 0
            tail_posts = attention(qi, pend, cv)
            while pend:
                pend.popleft()()
            carry = carry + tail_posts + out_dma_atoms(qi)
            if qi == NQ - 1:
                for a in carry:
                    a()

    nc.finalize()
    return nc


def get_nc():
    global _nc_cache
    if _nc_cache is None:
        _nc_cache = build_nc()
    return _nc_cache


def make_in_maps(x, pad_mask, Wq, bq, Wk, bk, Wv, bv):
    import ml_dtypes

    bf16 = ml_dtypes.bfloat16
    x = np.asarray(x, np.float32)
    pad_mask = np.asarray(pad_mask, np.float32)
    Wq = np.asarray(Wq, np.float32)
    bq = np.asarray(bq, np.float32)
    Wk = np.asarray(Wk, np.float32)
    bk = np.asarray(bk, np.float32)
    Wv = np.asarray(Wv, np.float32)
    bv = np.asarray(bv, np.float32)
    in_maps = []
    for c in range(NCORES):
        b, g = divmod(c, 4)
        cols = slice(g * C, (g + 1) * C)
        xt = np.ascontiguousarray(x[b].T).astype(bf16)  # [HIN, S]
        pn = ((pad_mask[b] - 1.0) * 1e6).reshape(NKT, P).T.copy()  # [P, NKT]
        in_maps.append(
            dict(
                xt=xt,
                wq=np.ascontiguousarray(Wq[:, cols]).astype(bf16),
                wk=np.ascontiguousarray(Wk[:, cols]).astype(bf16),
                wv=np.ascontiguousarray(Wv[:, cols]).astype(bf16),
                bqc=np.ascontiguousarray(bq[cols].reshape(NPAIR, P).T),
                bkc=np.ascontiguousarray(bk[cols].reshape(NPAIR, P).T),
                bv=np.ascontiguousarray(bv[cols].reshape(1, C)).astype(bf16),
                padneg=pn,
            )
        )
    return in_maps


def gather(results):
    B = 2
    out = np.empty((B, S, HIN), np.float32)
    kcache = np.empty((B, S, HIN), np.float32)
    vcache = np.empty((B, S, HIN), np.float32)
    for c in range(NCORES):
        b, g = divmod(c, 4)
        cols = slice(g * C, (g + 1) * C)
        out[b, :, cols] = results[c]["out"].astype(np.float32)
        kcache[b, :, cols] = results[c]["kct"].T.astype(np.float32)
        vcache[b, :, cols] = results[c]["vc"].astype(np.float32)
    return out, kcache, vcache


def kernel(x, pad_mask, Wq, bq, Wk, bk, Wv, bv):
    from concourse.bass_utils import run_bass_kernel_spmd

    nc = get_nc()
    in_maps = make_in_maps(x, pad_mask, Wq, bq, Wk, bk, Wv, bv)
    res = run_bass_kernel_spmd(nc, in_maps, list(range(NCORES)))
    return gather(res.results)
